# revision 30
# baseline (speedup 1.0000x reference)
"""Fused conv-attention kernel for Trainium2, sharded over 8 NeuronCores.

Reference computation (B=2, H=12, L=T=1024, D=64, FEA=3, DIM=768):
    scores = concat([s0,s1,s2], ch)            # [b, 36, l, t]
    fused  = einsum('bclt,oc->bolt', scores, fuse_w) + fuse_b
    attn   = softmax(fused, axis=-1)
    x      = einsum('bhlt,bhtd->bhld', attn, v)
    y      = merge_heads(x) @ proj_w.T + proj_b  # [b, l, 768]

Sharding: fully data-parallel over (b, l-block): core k handles b=k//4 and
l-rows [256*(k%4), 256*(k%4)+256).  Every op is local; no collectives.

Per-core dataflow:
  - scores are host-cast to bf16 and packed per core into sa[g, (lg,c),
    (j,t)] so each group of 8 l-rows loads with ONE contiguous 590KB DMA
    whose outermost dim is 96 partitions -> HWDGE stripes it over all 16
    SDMA engines (the balanced-AP outermost dim is what the hardware
    round-robins; a "c lg t -> lg c t" rearrange would make it lg=8 ->
    only 8 engines).  Weights/v/proj singles go on the second HWDGE ring
    (nc.scalar -> qActDynamicHW) so they don't queue ahead of the score
    stream on the sync ring.
  - conv as block-diag matmul: 8 l-rows per group, lhsT_j [96,96] holds
    fuse_w columns for score tensor j replicated block-diagonally; j-outer
    loop keeps each wt_j stationary across both PSUM halves (fewer
    LDWEIGHTS), K=96, M=96, N=512 per matmul, PSUM-accumulated over j.
  - exp via ScalarE activation (bias=fuse_b, accum_out=row sums); softmax max
    subtraction is skipped (|fused| <= ~5 so exp is safe in fp32).
  - normalize by 1/rowsum on VectorE; transpose 128-col chunks on PE into
    PSUM batched 4 t-tiles per bank, then one 4-D strided DVE copy per batch
    into attn^T [t, (tt, head, l)] layout.
  - attn @ V as out[d, l] = v^T-stationary matmuls (K=t tiles of 128, N=256);
    v is host-packed into the exact SBUF layout [t-part, (h, tt, d)] so it
    loads as one contiguous 3.1MB DMA with 24KB lines.
  - final proj as out[l, 768] = x^T-stationary matmuls, bias added by DVE.
Matmul operands are bitcast to float32r (fp32 data, 4x PE throughput).
"""

import os
import sys

import numpy as np

sys.path.insert(0, "/opt/trn_rl_repo")

B, H, L, T, D = 2, 12, 1024, 1024, 64
DIM = H * D  # 768
NCORES = 8
LC = L * B // NCORES  # 256 l-rows per core
G = 8  # l-rows per conv group
NG = LC // G  # 32 groups
KM = 12 * G  # 96: conv matmul K and M
NTT = T // 128  # 8 t-tiles

_CACHE = {}


def _build_nc():
    import concourse.bacc as bacc
    import concourse.bass as bass
    import concourse.mybir as mybir
    import concourse.tile as tile
    from concourse.masks import make_identity
    from contextlib import ExitStack

    f32 = mybir.dt.float32
    f32r = mybir.dt.float32r
    bf16 = mybir.dt.bfloat16

    nc = bacc.Bacc(
        "TRN2", target_bir_lowering=False, debug=False, enable_asserts=False
    )

    sa_in = nc.dram_tensor("sa", [NG, KM, 3 * T], bf16, kind="ExternalInput").ap()
    v_in = nc.dram_tensor("vc", [128, H * NTT * D], bf16, kind="ExternalInput").ap()
    w_in = [
        nc.dram_tensor(f"w{j}", [KM, KM], bf16, kind="ExternalInput").ap()
        for j in range(3)
    ]
    b_in = nc.dram_tensor("b96", [KM, 1], f32, kind="ExternalInput").ap()
    pw_in = nc.dram_tensor("pwT", [DIM, DIM], f32r, kind="ExternalInput").ap()
    pb_in = nc.dram_tensor("pbb", [128, DIM], f32, kind="ExternalInput").ap()
    out_d = nc.dram_tensor("out", [LC, DIM], f32, kind="ExternalOutput").ap()

    with tile.TileContext(nc) as tc, ExitStack() as ctx:
        # ---- persistent SBUF ----
        singles = ctx.enter_context(tc.tile_pool(name="singles", bufs=1))
        ident = singles.tile([128, 128], bf16)
        make_identity(nc, ident[:])
        wt = [singles.tile([KM, KM], bf16, tag=f"wt{j}", name=f"wt{j}") for j in range(3)]
        for j in range(3):
            nc.scalar.dma_start(wt[j][:], w_in[j])
        b96 = singles.tile([KM, 1], f32)
        nc.scalar.dma_start(b96[:], b_in)
        pw = singles.tile([128, 6 * DIM], f32r)  # [i-tile part, ki*768+o]
        pb = singles.tile([128, DIM], f32)
        vsb = singles.tile([128, H * NTT * D], bf16)  # [t-part, h*512 + tt*64 + d]
        # attn^T accumulator: [t-part(128), tt*3072 + h*256 + l]
        attnT = singles.tile([128, NTT * H * LC], bf16)
        # x^T for proj: [i%128 part, (i//128)*256 + l]
        xT = singles.tile([128, 6 * LC], f32r)

        # ---- phase 1: conv + softmax + transpose, per group of 8 l-rows ----
        with ExitStack() as p1:
            spool = p1.enter_context(tc.tile_pool(name="scores", bufs=4))
            fpsum = p1.enter_context(
                tc.tile_pool(name="fpsum", bufs=3, space="PSUM")
            )
            epool = p1.enter_context(tc.tile_pool(name="exp", bufs=3))
            zpool = p1.enter_context(tc.tile_pool(name="z", bufs=4))
            tpsum = p1.enter_context(
                tc.tile_pool(name="tpsum", bufs=2, space="PSUM")
            )
            attnT4 = attnT[:].rearrange("p (tt h l) -> p tt h l", tt=NTT, h=H)
            et_tiles = {}

            def emit_transpose(gp):
                # PE transpose all 8 t-tiles of group gp into one PSUM tile,
                # then one 4-D DVE copy: attnT[p, tt, o, gp*8+lg] from
                # tp[p, (tt, o, lg)] = et[(o,lg), tt*128+p].
                etp = et_tiles.pop(gp)
                tp = tpsum.tile([128, NTT * KM], bf16, tag="tp", name=f"tp_{gp}")
                for tt in range(NTT):
                    nc.tensor.transpose(
                        tp[:, tt * KM : (tt + 1) * KM],
                        etp[:, tt * 128 : (tt + 1) * 128],
                        ident[:KM, :KM],
                    )
                nc.vector.tensor_copy(
                    attnT4[:, :, :, gp * G : (gp + 1) * G],
                    tp[:].rearrange("p (tt o lg) -> p tt o lg", tt=NTT, o=H),
                )

            for g in range(NG):
                st = spool.tile([KM, 3 * T], bf16, tag="st", name=f"st_{g}")
                ring = nc.sync if g % 8 < 5 else nc.scalar
                ring.dma_start(st[:], sa_in[g])
                if g == 7:
                    nc.scalar.dma_start(vsb[:], v_in)
                if g == 15:
                    for ki in range(6):
                        nc.scalar.dma_start(
                            pw[:, ki * DIM : (ki + 1) * DIM],
                            pw_in[ki * 128 : (ki + 1) * 128, :],
                        )
                    nc.scalar.dma_start(pb[:], pb_in)
                fp = fpsum.tile([KM, T], f32)
                for j in range(3):
                    for th in range(2):
                        nc.tensor.matmul(
                            fp[:, th * 512 : (th + 1) * 512],
                            wt[j][:],
                            st[:, j * T + th * 512 : j * T + (th + 1) * 512],
                            start=(j == 0),
                            stop=(j == 2),
                        )
                # software-pipelined: previous group's transposes go after this
                # group's conv so the PE never waits on the exp/norm chain
                if g >= 1:
                    emit_transpose(g - 1)
                et = epool.tile([KM, T], bf16)
                et_tiles[g] = et
                zt = zpool.tile([KM, 1], f32, tag="zt")
                nc.scalar.activation(
                    et[:],
                    fp[:],
                    mybir.ActivationFunctionType.Exp,
                    bias=b96[:],
                    accum_out=zt[:],
                )
                zi = zpool.tile([KM, 1], f32, tag="zi")
                nc.vector.reciprocal(zi[:], zt[:])
                nc.vector.tensor_scalar_mul(et[:], et[:], zi[:])
            emit_transpose(NG - 1)

        # ---- phase 2+3: attn @ V -> x^T, proj folded in per head-pair ----
        with ExitStack() as p2:
            xpsum = p2.enter_context(
                tc.tile_pool(name="xpsum", bufs=2, space="PSUM")
            )
            ppsum = p2.enter_context(
                tc.tile_pool(name="ppsum", bufs=1, space="PSUM")
            )
            ypool = p2.enter_context(tc.tile_pool(name="y", bufs=2))
            pp = [
                ppsum.tile([128, 1024], f32, tag=f"pp{lc}", name=f"pp{lc}")
                for lc in range(2)
            ]

            def emit_proj(ki):
                for lc in range(2):
                    lhs = xT[:, ki * LC + lc * 128 : ki * LC + (lc + 1) * 128]
                    nc.tensor.matmul(
                        pp[lc][:, 0:512],
                        lhs,
                        pw[:, ki * DIM : ki * DIM + 512],
                        start=(ki == 0),
                        stop=(ki == 5),
                    )
                    nc.tensor.matmul(
                        pp[lc][:, 512:768],
                        lhs,
                        pw[:, ki * DIM + 512 : ki * DIM + DIM],
                        start=(ki == 0),
                        stop=(ki == 5),
                    )

            for h in range(H):
                xp = xpsum.tile([D, LC], f32)
                for tt in range(NTT):
                    nc.tensor.matmul(
                        xp[:],
                        vsb[:, h * 512 + tt * D : h * 512 + (tt + 1) * D],
                        attnT[
                            :, tt * H * LC + h * LC : tt * H * LC + (h + 1) * LC
                        ],
                        start=(tt == 0),
                        stop=(tt == NTT - 1),
                    )
                po = (h % 2) * D
                ko = (h // 2) * LC
                nc.vector.tensor_copy(xT[po : po + D, ko : ko + LC], xp[:])
                # proj for head-pair ki streams one head behind its xT copies
                if h >= 3 and h % 2 == 1:
                    emit_proj(h // 2 - 1)
            emit_proj(5)
            for lc in range(2):
                yt = ypool.tile([128, DIM], f32)
                nc.vector.tensor_add(yt[:], pp[lc][:, 0:DIM], pb[:])
                nc.sync.dma_start(out_d[lc * 128 : (lc + 1) * 128, :], yt[:])

    nc.compile()
    return nc


def _host_prep(s0, s1, s2, v, fuse_w, fuse_b, proj_w, proj_b):
    """Build per-core input maps."""
    s0 = np.asarray(s0, dtype=np.float32)
    s1 = np.asarray(s1, dtype=np.float32)
    s2 = np.asarray(s2, dtype=np.float32)
    v = np.asarray(v, dtype=np.float32)
    fuse_w = np.asarray(fuse_w, dtype=np.float32)
    fuse_b = np.asarray(fuse_b, dtype=np.float32)
    proj_w = np.asarray(proj_w, dtype=np.float32)
    proj_b = np.asarray(proj_b, dtype=np.float32)
    import ml_dtypes

    # block-diag conv weights: w_j[k=(lg,c), m=(o,lg)] = fuse_w[o, 12j+c] @ lg==lg'
    ws = []
    for j in range(3):
        wj = np.zeros((KM, KM), dtype=np.float32)
        blk = fuse_w[:, 12 * j : 12 * (j + 1)].T  # [c, o]
        for lg in range(G):
            # rows lg*12..lg*12+12 (c), cols o*G+lg
            wj[lg * 12 : (lg + 1) * 12, lg::G] = blk
        ws.append(wj)
    b96 = np.repeat(fuse_b, G).astype(np.float32).reshape(KM, 1)  # p = o*G+lg
    pwT = np.ascontiguousarray(proj_w.T)
    pbb = np.broadcast_to(proj_b, (128, DIM)).copy()

    bf16 = ml_dtypes.bfloat16

    # pack scores: sa[b, lblk][g, lg*12+c, j*T+t] = s_j[b, c, lblk*LC + g*G+lg, t]
    # [3, B, 12, L, T] -> [B, nb, NG, G, 12, 3, T] -> per core [NG, KM, 3T]
    s_all = np.stack([s0, s1, s2], axis=0).astype(bf16)
    nb = NCORES // B
    s_all = s_all.transpose(1, 2, 0, 3, 4).reshape(B, 12, 3, nb, NG, G, T)
    s_all = np.ascontiguousarray(s_all.transpose(0, 3, 4, 5, 1, 2, 6))
    s_all = s_all.reshape(B, nb, NG, KM, 3 * T)

    # pack v: vc[b][p, h*512 + tt*64 + d] = v[b, h, tt*128+p, d]
    vp = v.astype(bf16).reshape(B, H, NTT, 128, D).transpose(0, 3, 1, 2, 4)
    vp = np.ascontiguousarray(vp).reshape(B, 128, H * NTT * D)
    ws = [w.astype(bf16) for w in ws]

    in_maps = []
    for k in range(NCORES):
        b = k // nb
        lb = k % nb
        m = {
            "sa": s_all[b, lb],
            "vc": vp[b],
            "w0": ws[0],
            "w1": ws[1],
            "w2": ws[2],
            "b96": b96,
            "pwT": pwT,
            "pbb": pbb,
        }
        in_maps.append(m)
    return in_maps


def _install_ntff_hook():
    """Provide antenv.axon_hooks (absent in this image) so trace=True works."""
    try:
        from antenv import axon_hooks  # noqa: F401

        return True
    except ImportError:
        pass
    try:
        import types
        import ctypes
        import contextlib
        import antenv

        so_path = "/opt/axon/libaxon_pjrt.so"
        if not os.path.exists(so_path):
            return False
        lib = ctypes.CDLL(so_path)
        if not hasattr(lib, "axon_start_nrt_profile"):
            return False
        lib.axon_start_nrt_profile.argtypes = [
            ctypes.POINTER(ctypes.c_int64),
            ctypes.c_size_t,
        ]
        lib.axon_start_nrt_profile.restype = ctypes.c_int64
        lib.axon_stop_nrt_profile.argtypes = [ctypes.c_char_p]
        lib.axon_stop_nrt_profile.restype = ctypes.c_int64

        @contextlib.contextmanager
        def _hook(output_dir, device_ids):
            import jax

            jax.devices()
            if device_ids:
                ids = (ctypes.c_int64 * len(device_ids))(*device_ids)
                rc = lib.axon_start_nrt_profile(ids, len(device_ids))
            else:
                rc = lib.axon_start_nrt_profile(None, 0)
            if rc != 0:
                raise RuntimeError(f"axon_start_nrt_profile rc={rc}")
            try:
                yield
            finally:
                n = lib.axon_stop_nrt_profile(str(output_dir).encode())
                print(f"ntff profile: {n} file(s) -> {output_dir}", file=sys.stderr)

        mod = types.ModuleType("antenv.axon_hooks")
        _h = {"hook": _hook}
        mod.set_axon_ntff_profile_hook = lambda h: _h.__setitem__("hook", h)
        mod.get_axon_ntff_profile_hook = lambda: _h["hook"]
        sys.modules["antenv.axon_hooks"] = mod
        antenv.axon_hooks = mod
        return True
    except Exception as e:  # degrade to untraced
        print("ntff hook install failed:", e, file=sys.stderr)
        return False


def kernel(s0, s1, s2, v, fuse_w, fuse_b, proj_w, proj_b, _trace=False):
    from concourse import bass_utils
    from concourse.bass_utils import run_bass_kernel_spmd

    if "nc" not in _CACHE:
        _CACHE["nc"] = _build_nc()
    nc = _CACHE["nc"]

    in_maps = _host_prep(s0, s1, s2, v, fuse_w, fuse_b, proj_w, proj_b)
    if _trace:
        _trace = _install_ntff_hook()
        bass_utils.upload_artifacts = lambda tmpdir: f"local:{tmpdir}"
    tmpdir = None
    if _trace:
        import tempfile

        tmpdir = tempfile.mkdtemp(prefix="bass_trace_")
        _CACHE["trace_dir"] = tmpdir
    try:
        res = run_bass_kernel_spmd(
            nc, in_maps, core_ids=list(range(NCORES)), trace=_trace, tmpdir=tmpdir
        )
    except Exception:
        if not _trace:
            raise
        import traceback

        traceback.print_exc()
        print("trace run failed; retrying untraced", file=sys.stderr)
        res = run_bass_kernel_spmd(nc, in_maps, core_ids=list(range(NCORES)))
    _CACHE["last_exec_time_ns"] = res.exec_time_ns
    _CACHE["last_results"] = res

    out = np.empty((B, L, DIM), dtype=np.float32)
    for k in range(NCORES):
        b = k // (NCORES // B)
        l0 = (k % (NCORES // B)) * LC
        out[b, l0 : l0 + LC, :] = res.results[k]["out"]
    return out


# revision 31
# speedup vs baseline: 1.1251x; 1.1251x over previous
"""Fused conv-attention kernel for Trainium2, sharded over 8 NeuronCores.

Reference computation (B=2, H=12, L=T=1024, D=64, FEA=3, DIM=768):
    scores = concat([s0,s1,s2], ch)            # [b, 36, l, t]
    fused  = einsum('bclt,oc->bolt', scores, fuse_w) + fuse_b
    attn   = softmax(fused, axis=-1)
    x      = einsum('bhlt,bhtd->bhld', attn, v)
    y      = merge_heads(x) @ proj_w.T + proj_b  # [b, l, 768]

Sharding: fully data-parallel over (b, l-block): core k handles b=k//4 and
l-rows [256*(k%4), 256*(k%4)+256).  Every op is local; no collectives.

Per-core dataflow:
  - scores are host-cast to bf16 and packed per core into sa[g, (lg,c),
    (j,t)] so each group of 8 l-rows loads with ONE contiguous 590KB DMA
    whose outermost dim is 96 partitions -> HWDGE stripes it over all 16
    SDMA engines (the balanced-AP outermost dim is what the hardware
    round-robins; a "c lg t -> lg c t" rearrange would make it lg=8 ->
    only 8 engines).  Weights/v/proj singles go on the second HWDGE ring
    (nc.scalar -> qActDynamicHW) so they don't queue ahead of the score
    stream on the sync ring.
  - conv as block-diag matmul: 8 l-rows per group, lhsT_j [96,96] holds
    fuse_w columns for score tensor j replicated block-diagonally; j-outer
    loop keeps each wt_j stationary across both PSUM halves (fewer
    LDWEIGHTS), K=96, M=96, N=512 per matmul, PSUM-accumulated over j.
  - exp via ScalarE activation (bias=fuse_b, accum_out=row sums); softmax max
    subtraction is skipped (|fused| <= ~5 so exp is safe in fp32).
  - normalize by 1/rowsum on VectorE; transpose 128-col chunks on PE into
    PSUM batched 4 t-tiles per bank, then one 4-D strided DVE copy per batch
    into attn^T [t, (tt, head, l)] layout.
  - attn @ V as out[d, l] = v^T-stationary matmuls (K=t tiles of 128, N=256);
    v is host-packed into the exact SBUF layout [t-part, (h, tt, d)] so it
    loads as one contiguous 3.1MB DMA with 24KB lines.
  - final proj as out[l, 768] = x^T-stationary matmuls, bias added by DVE.
Matmul operands are bitcast to float32r (fp32 data, 4x PE throughput).
"""

import os
import sys

import numpy as np

sys.path.insert(0, "/opt/trn_rl_repo")

B, H, L, T, D = 2, 12, 1024, 1024, 64
DIM = H * D  # 768
NCORES = 8
LC = L * B // NCORES  # 256 l-rows per core
G = 8  # l-rows per conv group
NG = LC // G  # 32 groups
KM = 12 * G  # 96: conv matmul K and M
NTT = T // 128  # 8 t-tiles

_CACHE = {}


def _build_nc():
    import concourse.bacc as bacc
    import concourse.bass as bass
    import concourse.mybir as mybir
    import concourse.tile as tile
    from concourse.masks import make_identity
    from contextlib import ExitStack

    f32 = mybir.dt.float32
    f32r = mybir.dt.float32r
    bf16 = mybir.dt.bfloat16

    nc = bacc.Bacc(
        "TRN2", target_bir_lowering=False, debug=False, enable_asserts=False
    )

    sa_in = nc.dram_tensor("sa", [NG, KM, 3 * T], bf16, kind="ExternalInput").ap()
    v_in = nc.dram_tensor("vc", [128, H * NTT * D], bf16, kind="ExternalInput").ap()
    w_in = [
        nc.dram_tensor(f"w{j}", [KM, KM], bf16, kind="ExternalInput").ap()
        for j in range(3)
    ]
    b_in = nc.dram_tensor("b96", [KM, 1], f32, kind="ExternalInput").ap()
    pw_in = nc.dram_tensor("pwT", [DIM, DIM], f32r, kind="ExternalInput").ap()
    pb_in = nc.dram_tensor("pbb", [128, DIM], f32, kind="ExternalInput").ap()
    out_d = nc.dram_tensor("out", [LC, DIM], f32, kind="ExternalOutput").ap()

    with tile.TileContext(nc) as tc, ExitStack() as ctx:
        # ---- persistent SBUF ----
        singles = ctx.enter_context(tc.tile_pool(name="singles", bufs=1))
        ident = singles.tile([128, 128], bf16)
        make_identity(nc, ident[:])
        wt = [singles.tile([KM, KM], bf16, tag=f"wt{j}", name=f"wt{j}") for j in range(3)]
        for j in range(3):
            nc.scalar.dma_start(wt[j][:], w_in[j])
        b96 = singles.tile([KM, 1], f32)
        nc.scalar.dma_start(b96[:], b_in)
        pw = singles.tile([128, 6 * DIM], f32r)  # [i-tile part, ki*768+o]
        for ki in range(6):
            nc.scalar.dma_start(
                pw[:, ki * DIM : (ki + 1) * DIM], pw_in[ki * 128 : (ki + 1) * 128, :]
            )
        pb = singles.tile([128, DIM], f32)
        nc.scalar.dma_start(pb[:], pb_in)
        vsb = singles.tile([128, H * NTT * D], bf16)  # [t-part, h*512 + tt*64 + d]
        nc.scalar.dma_start(vsb[:], v_in)
        # attn^T accumulator: [t-part(128), tt*3072 + h*256 + l]
        attnT = singles.tile([128, NTT * H * LC], bf16)
        # x^T for proj: [i%128 part, (i//128)*256 + l]
        xT = singles.tile([128, 6 * LC], f32r)

        # ---- phase 1: conv + softmax + transpose, per group of 8 l-rows ----
        with ExitStack() as p1:
            spool = p1.enter_context(tc.tile_pool(name="scores", bufs=4))
            fpsum = p1.enter_context(
                tc.tile_pool(name="fpsum", bufs=3, space="PSUM")
            )
            epool = p1.enter_context(tc.tile_pool(name="exp", bufs=3))
            zpool = p1.enter_context(tc.tile_pool(name="z", bufs=4))
            tpsum = p1.enter_context(
                tc.tile_pool(name="tpsum", bufs=2, space="PSUM")
            )
            attnT4 = attnT[:].rearrange("p (tt h l) -> p tt h l", tt=NTT, h=H)
            et_tiles = {}

            def emit_transpose(gp):
                # PE transpose all 8 t-tiles of group gp into one PSUM tile,
                # then one 4-D DVE copy: attnT[p, tt, o, gp*8+lg] from
                # tp[p, (tt, o, lg)] = et[(o,lg), tt*128+p].
                etp = et_tiles.pop(gp)
                tp = tpsum.tile([128, NTT * KM], bf16, tag="tp", name=f"tp_{gp}")
                for tt in range(NTT):
                    nc.tensor.transpose(
                        tp[:, tt * KM : (tt + 1) * KM],
                        etp[:, tt * 128 : (tt + 1) * 128],
                        ident[:KM, :KM],
                    )
                nc.vector.tensor_copy(
                    attnT4[:, :, :, gp * G : (gp + 1) * G],
                    tp[:].rearrange("p (tt o lg) -> p tt o lg", tt=NTT, o=H),
                )

            for g in range(NG):
                st = spool.tile([KM, 3 * T], bf16, tag="st", name=f"st_{g}")
                nc.sync.dma_start(st[:], sa_in[g])
                fp = fpsum.tile([KM, T], f32)
                for j in range(3):
                    for th in range(2):
                        nc.tensor.matmul(
                            fp[:, th * 512 : (th + 1) * 512],
                            wt[j][:],
                            st[:, j * T + th * 512 : j * T + (th + 1) * 512],
                            start=(j == 0),
                            stop=(j == 2),
                        )
                # software-pipelined: previous group's transposes go after this
                # group's conv so the PE never waits on the exp/norm chain
                if g >= 1:
                    emit_transpose(g - 1)
                et = epool.tile([KM, T], bf16)
                et_tiles[g] = et
                zt = zpool.tile([KM, 1], f32, tag="zt")
                nc.scalar.activation(
                    et[:],
                    fp[:],
                    mybir.ActivationFunctionType.Exp,
                    bias=b96[:],
                    accum_out=zt[:],
                )
                zi = zpool.tile([KM, 1], f32, tag="zi")
                nc.vector.reciprocal(zi[:], zt[:])
                nc.vector.tensor_scalar_mul(et[:], et[:], zi[:])
            emit_transpose(NG - 1)

        # ---- phase 2+3: attn @ V -> x^T, proj folded in per head-pair ----
        with ExitStack() as p2:
            xpsum = p2.enter_context(
                tc.tile_pool(name="xpsum", bufs=2, space="PSUM")
            )
            ppsum = p2.enter_context(
                tc.tile_pool(name="ppsum", bufs=1, space="PSUM")
            )
            ypool = p2.enter_context(tc.tile_pool(name="y", bufs=2))
            pp = [
                ppsum.tile([128, 1024], f32, tag=f"pp{lc}", name=f"pp{lc}")
                for lc in range(2)
            ]

            def emit_proj(ki):
                for lc in range(2):
                    lhs = xT[:, ki * LC + lc * 128 : ki * LC + (lc + 1) * 128]
                    nc.tensor.matmul(
                        pp[lc][:, 0:512],
                        lhs,
                        pw[:, ki * DIM : ki * DIM + 512],
                        start=(ki == 0),
                        stop=(ki == 5),
                    )
                    nc.tensor.matmul(
                        pp[lc][:, 512:768],
                        lhs,
                        pw[:, ki * DIM + 512 : ki * DIM + DIM],
                        start=(ki == 0),
                        stop=(ki == 5),
                    )

            for h in range(H):
                xp = xpsum.tile([D, LC], f32)
                for tt in range(NTT):
                    nc.tensor.matmul(
                        xp[:],
                        vsb[:, h * 512 + tt * D : h * 512 + (tt + 1) * D],
                        attnT[
                            :, tt * H * LC + h * LC : tt * H * LC + (h + 1) * LC
                        ],
                        start=(tt == 0),
                        stop=(tt == NTT - 1),
                    )
                po = (h % 2) * D
                ko = (h // 2) * LC
                nc.vector.tensor_copy(xT[po : po + D, ko : ko + LC], xp[:])
                # proj for head-pair ki streams one head behind its xT copies
                if h >= 3 and h % 2 == 1:
                    emit_proj(h // 2 - 1)
            emit_proj(5)
            for lc in range(2):
                yt = ypool.tile([128, DIM], f32)
                nc.vector.tensor_add(yt[:], pp[lc][:, 0:DIM], pb[:])
                nc.sync.dma_start(out_d[lc * 128 : (lc + 1) * 128, :], yt[:])

    nc.compile()
    return nc


def _host_prep(s0, s1, s2, v, fuse_w, fuse_b, proj_w, proj_b):
    """Build per-core input maps."""
    s0 = np.asarray(s0, dtype=np.float32)
    s1 = np.asarray(s1, dtype=np.float32)
    s2 = np.asarray(s2, dtype=np.float32)
    v = np.asarray(v, dtype=np.float32)
    fuse_w = np.asarray(fuse_w, dtype=np.float32)
    fuse_b = np.asarray(fuse_b, dtype=np.float32)
    proj_w = np.asarray(proj_w, dtype=np.float32)
    proj_b = np.asarray(proj_b, dtype=np.float32)
    import ml_dtypes

    # block-diag conv weights: w_j[k=(lg,c), m=(o,lg)] = fuse_w[o, 12j+c] @ lg==lg'
    ws = []
    for j in range(3):
        wj = np.zeros((KM, KM), dtype=np.float32)
        blk = fuse_w[:, 12 * j : 12 * (j + 1)].T  # [c, o]
        for lg in range(G):
            # rows lg*12..lg*12+12 (c), cols o*G+lg
            wj[lg * 12 : (lg + 1) * 12, lg::G] = blk
        ws.append(wj)
    b96 = np.repeat(fuse_b, G).astype(np.float32).reshape(KM, 1)  # p = o*G+lg
    pwT = np.ascontiguousarray(proj_w.T)
    pbb = np.broadcast_to(proj_b, (128, DIM)).copy()

    bf16 = ml_dtypes.bfloat16

    # pack scores: sa[b, lblk][g, lg*12+c, j*T+t] = s_j[b, c, lblk*LC + g*G+lg, t]
    # [3, B, 12, L, T] -> [B, nb, NG, G, 12, 3, T] -> per core [NG, KM, 3T]
    s_all = np.stack([s0, s1, s2], axis=0).astype(bf16)
    nb = NCORES // B
    s_all = s_all.transpose(1, 2, 0, 3, 4).reshape(B, 12, 3, nb, NG, G, T)
    s_all = np.ascontiguousarray(s_all.transpose(0, 3, 4, 5, 1, 2, 6))
    s_all = s_all.reshape(B, nb, NG, KM, 3 * T)

    # pack v: vc[b][p, h*512 + tt*64 + d] = v[b, h, tt*128+p, d]
    vp = v.astype(bf16).reshape(B, H, NTT, 128, D).transpose(0, 3, 1, 2, 4)
    vp = np.ascontiguousarray(vp).reshape(B, 128, H * NTT * D)
    ws = [w.astype(bf16) for w in ws]

    in_maps = []
    for k in range(NCORES):
        b = k // nb
        lb = k % nb
        m = {
            "sa": s_all[b, lb],
            "vc": vp[b],
            "w0": ws[0],
            "w1": ws[1],
            "w2": ws[2],
            "b96": b96,
            "pwT": pwT,
            "pbb": pbb,
        }
        in_maps.append(m)
    return in_maps


def _install_ntff_hook():
    """Provide antenv.axon_hooks (absent in this image) so trace=True works."""
    try:
        from antenv import axon_hooks  # noqa: F401

        return True
    except ImportError:
        pass
    try:
        import types
        import ctypes
        import contextlib
        import antenv

        so_path = "/opt/axon/libaxon_pjrt.so"
        if not os.path.exists(so_path):
            return False
        lib = ctypes.CDLL(so_path)
        if not hasattr(lib, "axon_start_nrt_profile"):
            return False
        lib.axon_start_nrt_profile.argtypes = [
            ctypes.POINTER(ctypes.c_int64),
            ctypes.c_size_t,
        ]
        lib.axon_start_nrt_profile.restype = ctypes.c_int64
        lib.axon_stop_nrt_profile.argtypes = [ctypes.c_char_p]
        lib.axon_stop_nrt_profile.restype = ctypes.c_int64

        @contextlib.contextmanager
        def _hook(output_dir, device_ids):
            import jax

            jax.devices()
            if device_ids:
                ids = (ctypes.c_int64 * len(device_ids))(*device_ids)
                rc = lib.axon_start_nrt_profile(ids, len(device_ids))
            else:
                rc = lib.axon_start_nrt_profile(None, 0)
            if rc != 0:
                raise RuntimeError(f"axon_start_nrt_profile rc={rc}")
            try:
                yield
            finally:
                n = lib.axon_stop_nrt_profile(str(output_dir).encode())
                print(f"ntff profile: {n} file(s) -> {output_dir}", file=sys.stderr)

        mod = types.ModuleType("antenv.axon_hooks")
        _h = {"hook": _hook}
        mod.set_axon_ntff_profile_hook = lambda h: _h.__setitem__("hook", h)
        mod.get_axon_ntff_profile_hook = lambda: _h["hook"]
        sys.modules["antenv.axon_hooks"] = mod
        antenv.axon_hooks = mod
        return True
    except Exception as e:  # degrade to untraced
        print("ntff hook install failed:", e, file=sys.stderr)
        return False


def kernel(s0, s1, s2, v, fuse_w, fuse_b, proj_w, proj_b, _trace=False):
    from concourse import bass_utils
    from concourse.bass_utils import run_bass_kernel_spmd

    if "nc" not in _CACHE:
        _CACHE["nc"] = _build_nc()
    nc = _CACHE["nc"]

    in_maps = _host_prep(s0, s1, s2, v, fuse_w, fuse_b, proj_w, proj_b)
    if _trace:
        _trace = _install_ntff_hook()
        bass_utils.upload_artifacts = lambda tmpdir: f"local:{tmpdir}"
    tmpdir = None
    if _trace:
        import tempfile

        tmpdir = tempfile.mkdtemp(prefix="bass_trace_")
        _CACHE["trace_dir"] = tmpdir
    try:
        res = run_bass_kernel_spmd(
            nc, in_maps, core_ids=list(range(NCORES)), trace=_trace, tmpdir=tmpdir
        )
    except Exception:
        if not _trace:
            raise
        import traceback

        traceback.print_exc()
        print("trace run failed; retrying untraced", file=sys.stderr)
        res = run_bass_kernel_spmd(nc, in_maps, core_ids=list(range(NCORES)))
    _CACHE["last_exec_time_ns"] = res.exec_time_ns
    _CACHE["last_results"] = res

    out = np.empty((B, L, DIM), dtype=np.float32)
    for k in range(NCORES):
        b = k // (NCORES // B)
        l0 = (k % (NCORES // B)) * LC
        out[b, l0 : l0 + LC, :] = res.results[k]["out"]
    return out


# revision 33
# speedup vs baseline: 1.1532x; 1.0250x over previous
"""Fused conv-attention kernel for Trainium2, sharded over 8 NeuronCores.

Reference computation (B=2, H=12, L=T=1024, D=64, FEA=3, DIM=768):
    scores = concat([s0,s1,s2], ch)            # [b, 36, l, t]
    fused  = einsum('bclt,oc->bolt', scores, fuse_w) + fuse_b
    attn   = softmax(fused, axis=-1)
    x      = einsum('bhlt,bhtd->bhld', attn, v)
    y      = merge_heads(x) @ proj_w.T + proj_b  # [b, l, 768]

Sharding: fully data-parallel over (b, l-block): core k handles b=k//4 and
l-rows [256*(k%4), 256*(k%4)+256).  Every op is local; no collectives.

Per-core dataflow:
  - scores are host-cast to bf16 and packed per core into sa[g, (lg,c),
    (j,t)] so each group of 8 l-rows loads with ONE contiguous 590KB DMA
    whose outermost dim is 96 partitions -> HWDGE stripes it over all 16
    SDMA engines (the balanced-AP outermost dim is what the hardware
    round-robins; a "c lg t -> lg c t" rearrange would make it lg=8 ->
    only 8 engines).  Weights/v/proj singles go on the second HWDGE ring
    (nc.scalar -> qActDynamicHW) so they don't queue ahead of the score
    stream on the sync ring.
  - conv as block-diag matmul: 8 l-rows per group, lhsT_j [96,96] holds
    fuse_w columns for score tensor j replicated block-diagonally; j-outer
    loop keeps each wt_j stationary across both PSUM halves (fewer
    LDWEIGHTS), K=96, M=96, N=512 per matmul, PSUM-accumulated over j.
  - exp via ScalarE activation (bias=fuse_b, accum_out=row sums); softmax max
    subtraction is skipped (|fused| <= ~5 so exp is safe in fp32).
  - normalize by 1/rowsum on VectorE; software-pipelined PE transposes
    (group g-1's 8 t-tiles emitted after group g's conv so the PE never
    stalls on the exp/norm chain) into one PSUM tile, then a single 4-D
    strided DVE copy into attn^T [t, (tt, head, l)] layout.
  - attn @ V as out[d, l] = v^T-stationary matmuls (K=t tiles of 128, N=256);
    v is host-packed into the exact SBUF layout [t-part, (h, tt, d)] so it
    loads as one contiguous 3.1MB DMA with 24KB lines.
  - final proj as out[l, 768] = x^T-stationary matmuls, bias added by DVE.
Scores/v/attn run in bf16 (host-cast; ~3e-3 rel err vs 2e-2 gate); the
final projection stays float32r.
"""

import os
import sys

import numpy as np

sys.path.insert(0, "/opt/trn_rl_repo")

B, H, L, T, D = 2, 12, 1024, 1024, 64
DIM = H * D  # 768
NCORES = 8
LC = L * B // NCORES  # 256 l-rows per core
G = 8  # l-rows per conv group
NG = LC // G  # 32 groups
KM = 12 * G  # 96: conv matmul K and M
NTT = T // 128  # 8 t-tiles

_CACHE = {}


def _build_nc():
    import concourse.bacc as bacc
    import concourse.bass as bass
    import concourse.mybir as mybir
    import concourse.tile as tile
    from concourse.masks import make_identity
    from contextlib import ExitStack

    f32 = mybir.dt.float32
    f32r = mybir.dt.float32r
    bf16 = mybir.dt.bfloat16

    nc = bacc.Bacc(
        "TRN2", target_bir_lowering=False, debug=False, enable_asserts=False
    )

    sa_in = nc.dram_tensor("sa", [NG, KM, 3 * T], bf16, kind="ExternalInput").ap()
    v_in = nc.dram_tensor("vc", [128, H * NTT * D], bf16, kind="ExternalInput").ap()
    w_in = [
        nc.dram_tensor(f"w{j}", [KM, KM], bf16, kind="ExternalInput").ap()
        for j in range(3)
    ]
    b_in = nc.dram_tensor("b96", [KM, 1], f32, kind="ExternalInput").ap()
    pw_in = nc.dram_tensor("pwT", [DIM, DIM], f32r, kind="ExternalInput").ap()
    pb_in = nc.dram_tensor("pbb", [128, DIM], f32, kind="ExternalInput").ap()
    out_d = nc.dram_tensor("out", [LC, DIM], f32, kind="ExternalOutput").ap()

    with tile.TileContext(nc) as tc, ExitStack() as ctx:
        # ---- persistent SBUF ----
        singles = ctx.enter_context(tc.tile_pool(name="singles", bufs=1))
        ident = singles.tile([128, 128], bf16)
        make_identity(nc, ident[:])
        wt = [singles.tile([KM, KM], bf16, tag=f"wt{j}", name=f"wt{j}") for j in range(3)]
        for j in range(3):
            nc.scalar.dma_start(wt[j][:], w_in[j])
        b96 = singles.tile([KM, 1], f32)
        nc.scalar.dma_start(b96[:], b_in)
        pw = singles.tile([128, 6 * DIM], f32r)  # [i-tile part, ki*768+o]
        for ki in range(6):
            nc.scalar.dma_start(
                pw[:, ki * DIM : (ki + 1) * DIM], pw_in[ki * 128 : (ki + 1) * 128, :]
            )
        pb = singles.tile([128, DIM], f32)
        nc.scalar.dma_start(pb[:], pb_in)
        vsb = singles.tile([128, H * NTT * D], bf16)  # [t-part, h*512 + tt*64 + d]
        nc.scalar.dma_start(vsb[:], v_in)
        # attn^T accumulator: [t-part(128), tt*3072 + h*256 + l]
        attnT = singles.tile([128, NTT * H * LC], bf16)
        # x^T for proj: [i%128 part, (i//128)*256 + l]
        xT = singles.tile([128, 6 * LC], f32r)

        # ---- phase 1: conv + softmax + transpose, per group of 8 l-rows ----
        with ExitStack() as p1:
            spool = p1.enter_context(tc.tile_pool(name="scores", bufs=4))
            fpsum = p1.enter_context(
                tc.tile_pool(name="fpsum", bufs=3, space="PSUM")
            )
            epool = p1.enter_context(tc.tile_pool(name="exp", bufs=3))
            zpool = p1.enter_context(tc.tile_pool(name="z", bufs=4))
            tpsum = p1.enter_context(
                tc.tile_pool(name="tpsum", bufs=2, space="PSUM")
            )
            attnT4 = attnT[:].rearrange("p (tt h l) -> p tt h l", tt=NTT, h=H)
            et_tiles = {}

            def emit_transpose(gp):
                # PE transpose all 8 t-tiles of group gp into one PSUM tile,
                # then one 4-D DVE copy: attnT[p, tt, o, gp*8+lg] from
                # tp[p, (tt, o, lg)] = et[(o,lg), tt*128+p].
                etp = et_tiles.pop(gp)
                tp = tpsum.tile([128, NTT * KM], bf16, tag="tp", name=f"tp_{gp}")
                for tt in range(NTT):
                    nc.tensor.transpose(
                        tp[:, tt * KM : (tt + 1) * KM],
                        etp[:, tt * 128 : (tt + 1) * 128],
                        ident[:KM, :KM],
                    )
                nc.vector.tensor_copy(
                    attnT4[:, :, :, gp * G : (gp + 1) * G],
                    tp[:].rearrange("p (tt o lg) -> p tt o lg", tt=NTT, o=H),
                )

            for g in range(NG):
                st = spool.tile([KM, 3 * T], bf16, tag="st", name=f"st_{g}")
                nc.sync.dma_start(st[:], sa_in[g])
                fp = fpsum.tile([KM, T], f32)
                for j in range(3):
                    for th in range(2):
                        nc.tensor.matmul(
                            fp[:, th * 512 : (th + 1) * 512],
                            wt[j][:],
                            st[:, j * T + th * 512 : j * T + (th + 1) * 512],
                            start=(j == 0),
                            stop=(j == 2),
                        )
                # software-pipelined: previous group's transposes go after this
                # group's conv so the PE never waits on the exp/norm chain
                if g >= 1:
                    emit_transpose(g - 1)
                et = epool.tile([KM, T], bf16)
                et_tiles[g] = et
                zt = zpool.tile([KM, 1], f32, tag="zt")
                nc.scalar.activation(
                    et[:],
                    fp[:],
                    mybir.ActivationFunctionType.Exp,
                    bias=b96[:],
                    accum_out=zt[:],
                )
                zi = zpool.tile([KM, 1], f32, tag="zi")
                nc.vector.reciprocal(zi[:], zt[:])
                nc.vector.tensor_scalar_mul(et[:], et[:], zi[:])
            emit_transpose(NG - 1)

        # ---- phase 2: attn @ V  -> x^T ----
        with ExitStack() as p2:
            xpsum = p2.enter_context(
                tc.tile_pool(name="xpsum", bufs=2, space="PSUM")
            )
            for h in range(H):
                xp = xpsum.tile([D, LC], f32)
                for tt in range(NTT):
                    nc.tensor.matmul(
                        xp[:],
                        vsb[:, h * 512 + tt * D : h * 512 + (tt + 1) * D],
                        attnT[
                            :, tt * H * LC + h * LC : tt * H * LC + (h + 1) * LC
                        ],
                        start=(tt == 0),
                        stop=(tt == NTT - 1),
                    )
                po = (h % 2) * D
                ko = (h // 2) * LC
                nc.vector.tensor_copy(xT[po : po + D, ko : ko + LC], xp[:])

            # ---- phase 3: proj -> out ----
            ppsum = p2.enter_context(
                tc.tile_pool(name="ppsum", bufs=2, space="PSUM")
            )
            ypool = p2.enter_context(tc.tile_pool(name="y", bufs=2))
            for lc in range(2):
                pp = ppsum.tile([128, 1024], f32)
                for ki in range(6):
                    lhs = xT[:, ki * LC + lc * 128 : ki * LC + (lc + 1) * 128]
                    nc.tensor.matmul(
                        pp[:, 0:512],
                        lhs,
                        pw[:, ki * DIM : ki * DIM + 512],
                        start=(ki == 0),
                        stop=(ki == 5),
                    )
                    nc.tensor.matmul(
                        pp[:, 512:768],
                        lhs,
                        pw[:, ki * DIM + 512 : ki * DIM + DIM],
                        start=(ki == 0),
                        stop=(ki == 5),
                    )
                yt = ypool.tile([128, DIM], f32)
                nc.vector.tensor_add(yt[:], pp[:, 0:DIM], pb[:])
                nc.sync.dma_start(out_d[lc * 128 : (lc + 1) * 128, :], yt[:])

    nc.compile()
    return nc


def _host_prep(s0, s1, s2, v, fuse_w, fuse_b, proj_w, proj_b):
    """Build per-core input maps."""
    s0 = np.asarray(s0, dtype=np.float32)
    s1 = np.asarray(s1, dtype=np.float32)
    s2 = np.asarray(s2, dtype=np.float32)
    v = np.asarray(v, dtype=np.float32)
    fuse_w = np.asarray(fuse_w, dtype=np.float32)
    fuse_b = np.asarray(fuse_b, dtype=np.float32)
    proj_w = np.asarray(proj_w, dtype=np.float32)
    proj_b = np.asarray(proj_b, dtype=np.float32)
    import ml_dtypes

    # block-diag conv weights: w_j[k=(lg,c), m=(o,lg)] = fuse_w[o, 12j+c] @ lg==lg'
    ws = []
    for j in range(3):
        wj = np.zeros((KM, KM), dtype=np.float32)
        blk = fuse_w[:, 12 * j : 12 * (j + 1)].T  # [c, o]
        for lg in range(G):
            # rows lg*12..lg*12+12 (c), cols o*G+lg
            wj[lg * 12 : (lg + 1) * 12, lg::G] = blk
        ws.append(wj)
    b96 = np.repeat(fuse_b, G).astype(np.float32).reshape(KM, 1)  # p = o*G+lg
    pwT = np.ascontiguousarray(proj_w.T)
    pbb = np.broadcast_to(proj_b, (128, DIM)).copy()

    bf16 = ml_dtypes.bfloat16

    # pack scores: sa[b, lblk][g, lg*12+c, j*T+t] = s_j[b, c, lblk*LC + g*G+lg, t]
    # [3, B, 12, L, T] -> [B, nb, NG, G, 12, 3, T] -> per core [NG, KM, 3T]
    s_all = np.stack([s0, s1, s2], axis=0).astype(bf16)
    nb = NCORES // B
    s_all = s_all.transpose(1, 2, 0, 3, 4).reshape(B, 12, 3, nb, NG, G, T)
    s_all = np.ascontiguousarray(s_all.transpose(0, 3, 4, 5, 1, 2, 6))
    s_all = s_all.reshape(B, nb, NG, KM, 3 * T)

    # pack v: vc[b][p, h*512 + tt*64 + d] = v[b, h, tt*128+p, d]
    vp = v.astype(bf16).reshape(B, H, NTT, 128, D).transpose(0, 3, 1, 2, 4)
    vp = np.ascontiguousarray(vp).reshape(B, 128, H * NTT * D)
    ws = [w.astype(bf16) for w in ws]

    in_maps = []
    for k in range(NCORES):
        b = k // nb
        lb = k % nb
        m = {
            "sa": s_all[b, lb],
            "vc": vp[b],
            "w0": ws[0],
            "w1": ws[1],
            "w2": ws[2],
            "b96": b96,
            "pwT": pwT,
            "pbb": pbb,
        }
        in_maps.append(m)
    return in_maps


def _install_ntff_hook():
    """Provide antenv.axon_hooks (absent in this image) so trace=True works."""
    try:
        from antenv import axon_hooks  # noqa: F401

        return True
    except ImportError:
        pass
    try:
        import types
        import ctypes
        import contextlib
        import antenv

        so_path = "/opt/axon/libaxon_pjrt.so"
        if not os.path.exists(so_path):
            return False
        lib = ctypes.CDLL(so_path)
        if not hasattr(lib, "axon_start_nrt_profile"):
            return False
        lib.axon_start_nrt_profile.argtypes = [
            ctypes.POINTER(ctypes.c_int64),
            ctypes.c_size_t,
        ]
        lib.axon_start_nrt_profile.restype = ctypes.c_int64
        lib.axon_stop_nrt_profile.argtypes = [ctypes.c_char_p]
        lib.axon_stop_nrt_profile.restype = ctypes.c_int64

        @contextlib.contextmanager
        def _hook(output_dir, device_ids):
            import jax

            jax.devices()
            if device_ids:
                ids = (ctypes.c_int64 * len(device_ids))(*device_ids)
                rc = lib.axon_start_nrt_profile(ids, len(device_ids))
            else:
                rc = lib.axon_start_nrt_profile(None, 0)
            if rc != 0:
                raise RuntimeError(f"axon_start_nrt_profile rc={rc}")
            try:
                yield
            finally:
                n = lib.axon_stop_nrt_profile(str(output_dir).encode())
                print(f"ntff profile: {n} file(s) -> {output_dir}", file=sys.stderr)

        mod = types.ModuleType("antenv.axon_hooks")
        _h = {"hook": _hook}
        mod.set_axon_ntff_profile_hook = lambda h: _h.__setitem__("hook", h)
        mod.get_axon_ntff_profile_hook = lambda: _h["hook"]
        sys.modules["antenv.axon_hooks"] = mod
        antenv.axon_hooks = mod
        return True
    except Exception as e:  # degrade to untraced
        print("ntff hook install failed:", e, file=sys.stderr)
        return False


def kernel(s0, s1, s2, v, fuse_w, fuse_b, proj_w, proj_b, _trace=False):
    from concourse import bass_utils
    from concourse.bass_utils import run_bass_kernel_spmd

    if "nc" not in _CACHE:
        _CACHE["nc"] = _build_nc()
    nc = _CACHE["nc"]

    in_maps = _host_prep(s0, s1, s2, v, fuse_w, fuse_b, proj_w, proj_b)
    if _trace:
        _trace = _install_ntff_hook()
        bass_utils.upload_artifacts = lambda tmpdir: f"local:{tmpdir}"
    tmpdir = None
    if _trace:
        import tempfile

        tmpdir = tempfile.mkdtemp(prefix="bass_trace_")
        _CACHE["trace_dir"] = tmpdir
    try:
        res = run_bass_kernel_spmd(
            nc, in_maps, core_ids=list(range(NCORES)), trace=_trace, tmpdir=tmpdir
        )
    except Exception:
        if not _trace:
            raise
        import traceback

        traceback.print_exc()
        print("trace run failed; retrying untraced", file=sys.stderr)
        res = run_bass_kernel_spmd(nc, in_maps, core_ids=list(range(NCORES)))
    _CACHE["last_exec_time_ns"] = res.exec_time_ns
    _CACHE["last_results"] = res

    out = np.empty((B, L, DIM), dtype=np.float32)
    for k in range(NCORES):
        b = k // (NCORES // B)
        l0 = (k % (NCORES // B)) * LC
        out[b, l0 : l0 + LC, :] = res.results[k]["out"]
    return out
